# revision 1
# baseline (speedup 1.0000x reference)
"""Trainium2 Bass kernel for nn_DinoGazeSpade (segment_reduce + repaint).

reference semantics:
  seg_feat = mask[:, ::14, ::14]                       # nearest-downsample to 28x28
  seg_avg[b, s, :] = mean of feat pixels with seg==s   # scatter_mean over B*128 segments
  out[b, :, hi, wi] = seg_avg[b, mask[b, hi, wi], :]   # repaint at full res

Sharding: 8 cores = 2 batches x 4 row-slices of the 392-row full-res output.
Each core computes its batch's seg_avg table (tiny) and paints its 98-row
slice. The paint is a one-hot(segment) x seg_avg matmul on the tensor engine,
which directly produces the channel-major output layout. seg_avg is split
hi/lo into two bf16 matrices so two bf16 matmuls reproduce fp32 accuracy.
Features are shipped as exact bf16 hi/lo planes so the scatter-sum matmuls
also run at bf16 rate while accumulating the exact fp32 values.
"""

import numpy as np
import ml_dtypes
from contextlib import ExitStack

import concourse.bass as bass
import concourse.tile as tile
from concourse import bacc, mybir
from concourse.bass_utils import run_bass_kernel_spmd

# problem shape (hardcoded per contract)
B, C, Hp, Wp = 2, 768, 28, 28
Hi, Wi = 392, 392
S = 128                    # segments per image
N_CORES = 8
ROWS = Hi // 4             # 98 full-res rows per core
NPIX = ROWS * Wi           # 38416 pixels per core
NPATCH = Hp * Wp           # 784 patch pixels
PCHUNK = 112               # 784 = 7 * 112 patch-pixel chunks (partition dim)
PTILE = 512                # paint pixel tile (one PSUM bank)
GROUP = 3 * PTILE          # 1536 pixels per paint group
NGROUP = NPIX // GROUP     # 25 full groups
REM = NPIX - NGROUP * GROUP  # 16 remainder pixels
CT = C // 128              # 6 channel tiles

f32 = mybir.dt.float32
bf16 = mybir.dt.bfloat16
i32 = mybir.dt.int32

_CACHED_NC = None


def _build_nc():
    nc = bacc.Bacc()
    NCH = NPATCH // PCHUNK
    fpk_hbm = nc.dram_tensor("fpk", [PCHUNK, NCH, 2, C], bf16, kind="ExternalInput")
    pmk_hbm = nc.dram_tensor("pmk", [PCHUNK, NCH], f32, kind="ExternalInput")
    mask_hbm = nc.dram_tensor("mask", [1, NPIX], bf16, kind="ExternalInput")
    out_hbm = nc.dram_tensor("out", [C, NPIX], f32, kind="ExternalOutput")

    with tile.TileContext(nc) as tc, ExitStack() as ctx:
        const = ctx.enter_context(tc.tile_pool(name="const", bufs=1))
        segp = ctx.enter_context(tc.tile_pool(name="segp", bufs=1))
        # paint-phase SBUF pools created BEFORE the scatter scratch pool so
        # the scatter pool's release doesn't alias them (early mask loads can
        # then overlap the scatter phase)
        sbB = ctx.enter_context(tc.tile_pool(name="sbB", bufs=6))
        osb = ctx.enter_context(tc.tile_pool(name="osb", bufs=10))

        # ---- constants ----
        iota_pi = const.tile([128, 1], i32)           # partition index
        nc.gpsimd.iota(iota_pi[:], [[0, 1]], channel_multiplier=1)
        iota_pf = const.tile([128, 1], f32)
        nc.vector.tensor_copy(iota_pf[:], iota_pi[:])
        iota_ri = const.tile([128, 128], i32)         # free-dim index (same per partition)
        nc.gpsimd.iota(iota_ri[:], [[1, 128]], channel_multiplier=0)
        iota_rf = const.tile([128, 128], f32)
        nc.vector.tensor_copy(iota_rf[:], iota_ri[:])
        ones_bf = const.tile([1, 128], bf16)
        nc.vector.memset(ones_bf[:], 1.0)
        ones_col = const.tile([128, 1], bf16)
        nc.vector.memset(ones_col[:], 1.0)

        # ---- phase A: scatter-mean over patch pixels -> seg_avg [S=128, C] ----
        seg_sb = segp.tile([128, C], f32)
        hi_bf = segp.tile([128, C], bf16)
        lo_bf = segp.tile([128, C], bf16)

        psA_cm = tc.tile_pool(name="psA", bufs=1, space="PSUM")
        with tc.tile_pool(name="sbA", bufs=2) as sbA, psA_cm as psA:
            sums0 = psA.tile([128, 384], f32, tag="sums0", name="sums0")
            sums1 = psA.tile([128, 384], f32, tag="sums1", name="sums1")
            cnt_ps = psA.tile([128, 1], f32, tag="cnt", name="cnt")
            nchunk = NPATCH // PCHUNK
            fsb = sbA.tile([PCHUNK, nchunk, 2, C], bf16, tag="fsb")
            nc.gpsimd.dma_start(out=fsb[:], in_=fpk_hbm[:, :, :, :])
            pmk = sbA.tile([PCHUNK, nchunk], f32, tag="pmk")
            nc.gpsimd.dma_start(out=pmk[:], in_=pmk_hbm[:, :])
            for k in range(nchunk):
                oh = sbA.tile([PCHUNK, 128], bf16, tag="ohp")
                nc.vector.tensor_tensor(
                    out=oh[:], in0=iota_rf[0:PCHUNK, :],
                    in1=pmk[:, k:k + 1].to_broadcast([PCHUNK, 128]),
                    op=mybir.AluOpType.is_equal,
                )
                first, last = k == 0, k == nchunk - 1
                for half, ps in ((0, sums0), (1, sums1)):
                    sl = slice(half * 384, (half + 1) * 384)
                    nc.tensor.matmul(ps[:], lhsT=oh[:], rhs=fsb[:, k, 0, sl],
                                     start=first, stop=False)
                    nc.tensor.matmul(ps[:], lhsT=oh[:], rhs=fsb[:, k, 1, sl],
                                     start=False, stop=last)
                nc.tensor.matmul(cnt_ps[:], lhsT=oh[:], rhs=ones_col[0:PCHUNK, :],
                                 start=first, stop=last)

            # r = 1 / max(cnt, 1); empty segments have sums == 0 so avg == 0
            cnt_sb = sbA.tile([128, 1], f32)
            nc.vector.tensor_scalar_max(cnt_sb[:], cnt_ps[:], 1.0)
            rcp = sbA.tile([128, 1], f32)
            nc.vector.reciprocal(rcp[:], cnt_sb[:])
            nc.vector.tensor_scalar(
                out=seg_sb[:, 0:384], in0=sums0[:], scalar1=rcp[:], scalar2=None,
                op0=mybir.AluOpType.mult,
            )
            nc.vector.tensor_scalar(
                out=seg_sb[:, 384:768], in0=sums1[:], scalar1=rcp[:], scalar2=None,
                op0=mybir.AluOpType.mult,
            )
            # hi/lo bf16 split: seg = hi + lo to ~fp32 accuracy
            nc.vector.tensor_copy(hi_bf[:], seg_sb[:])
            hi_f = sbA.tile([128, C], f32)
            nc.vector.tensor_copy(hi_f[:], hi_bf[:])
            lo_f = sbA.tile([128, C], f32)
            nc.vector.tensor_sub(lo_f[:], seg_sb[:], hi_f[:])
            nc.vector.tensor_copy(lo_bf[:], lo_f[:])

        # ---- phase B: paint full-res pixels ----
        psB = ctx.enter_context(tc.tile_pool(name="psB", bufs=2, space="PSUM"))
        psO = ctx.enter_context(tc.tile_pool(name="psO", bufs=6, space="PSUM"))

        def paint(pix0, sizes):
            # one group: pixels [pix0, pix0+sum(sizes)), one tile per size
            npx = sum(sizes)
            offs = [sum(sizes[:t]) for t in range(len(sizes))]
            mch_bf = sbB.tile([1, npx], bf16, tag="mchb", name="mchb")
            nc.gpsimd.dma_start(out=mch_bf[:], in_=mask_hbm[0:1, pix0:pix0 + npx])
            ohs = []
            for t, sz in enumerate(sizes):
                bc = psB.tile([128, sz], f32, tag="bc", name="bc")
                nc.tensor.matmul(
                    bc[:], lhsT=ones_bf[:],
                    rhs=mch_bf[0:1, offs[t]:offs[t] + sz],
                    start=True, stop=True,
                )
                oh = sbB.tile([128, sz], bf16, tag="ohb", name="ohb")
                nc.vector.tensor_scalar(
                    out=oh[:], in0=bc[:], scalar1=iota_pf[:], scalar2=None,
                    op0=mybir.AluOpType.is_equal,
                )
                ohs.append(oh)
            for c in range(CT):
                ob = osb.tile([128, npx], f32, tag="ob", name="ob")
                ops = [psO.tile([128, sz], f32, tag="op", name="op")
                       for sz in sizes]
                # hi pass then lo pass over all tiles: the stationary
                # weight reloads only twice per channel tile
                for t in range(len(sizes)):
                    nc.tensor.matmul(ops[t][:], lhsT=hi_bf[:, c * 128:(c + 1) * 128],
                                     rhs=ohs[t][:], start=True, stop=False)
                for t in range(len(sizes)):
                    nc.tensor.matmul(ops[t][:], lhsT=lo_bf[:, c * 128:(c + 1) * 128],
                                     rhs=ohs[t][:], start=False, stop=True)
                for t in range(len(sizes)):
                    # split psum->sbuf copies across DVE and ACT
                    dst = ob[:, offs[t]:offs[t] + sizes[t]]
                    if (c * len(sizes) + t) % 2 == 0:
                        nc.vector.tensor_copy(dst, ops[t][:])
                    else:
                        nc.scalar.copy(dst, ops[t][:])
                nc.sync.dma_start(
                    out=out_hbm[c * 128:(c + 1) * 128, pix0:pix0 + npx], in_=ob[:]
                )

        for g in range(NGROUP - 1):
            paint(g * GROUP, [PTILE] * 3)
        # last group absorbs the 16-pixel remainder as a 4th tile so the
        # final output DMA stays one large contiguous transfer per c-tile
        paint((NGROUP - 1) * GROUP, [PTILE] * 3 + ([REM] if REM else []))

    nc.compile()
    return nc


def _split_hilo(x):
    hi = x.astype(ml_dtypes.bfloat16)
    lo = (x - hi.astype(np.float32)).astype(ml_dtypes.bfloat16)
    return hi, lo


def make_in_maps(F_semantic_patches, segmentation_mask):
    F = np.asarray(F_semantic_patches, dtype=np.float32)
    M = np.asarray(segmentation_mask)
    in_maps = []
    for core in range(N_CORES):
        b, q = divmod(core, 4)
        nch = NPATCH // PCHUNK
        feat = F[b].reshape(C, NPATCH).T                               # [784, 768]
        fhi, flo = _split_hilo(feat)
        # [p, k, plane, c] so one DMA lands chunk k on partitions
        fpk = np.ascontiguousarray(
            np.stack([fhi.reshape(nch, PCHUNK, C), flo.reshape(nch, PCHUNK, C)],
                     axis=2).transpose(1, 0, 2, 3)
        )
        pmk = np.ascontiguousarray(
            M[b, ::Hi // Hp, ::Wi // Wp].reshape(nch, PCHUNK).T
        ).astype(np.float32)
        mask = np.ascontiguousarray(
            M[b, q * ROWS:(q + 1) * ROWS, :].reshape(1, NPIX)
        ).astype(ml_dtypes.bfloat16)
        in_maps.append({"fpk": fpk, "pmk": pmk, "mask": mask})
    return in_maps


def kernel(F_semantic_patches: np.ndarray, segmentation_mask: np.ndarray) -> np.ndarray:
    global _CACHED_NC
    if _CACHED_NC is None:
        _CACHED_NC = _build_nc()
    nc = _CACHED_NC

    in_maps = make_in_maps(F_semantic_patches, segmentation_mask)

    res = run_bass_kernel_spmd(nc, in_maps, core_ids=list(range(N_CORES)))

    out = np.empty((B, C, Hi, Wi), dtype=np.float32)
    for core in range(N_CORES):
        b, q = divmod(core, 4)
        out[b, :, q * ROWS:(q + 1) * ROWS, :] = (
            res.results[core]["out"].reshape(C, ROWS, Wi)
        )
    return out



# revision 2
# speedup vs baseline: 1.0380x; 1.0380x over previous
"""Trainium2 Bass kernel for nn_DinoGazeSpade (segment_reduce + repaint).

reference semantics:
  seg_feat = mask[:, ::14, ::14]                       # nearest-downsample to 28x28
  seg_avg[b, s, :] = mean of feat pixels with seg==s   # scatter_mean over B*128 segments
  out[b, :, hi, wi] = seg_avg[b, mask[b, hi, wi], :]   # repaint at full res

Sharding: 8 cores = 2 batches x 4 row-slices of the 392-row full-res output.
Each core computes its batch's seg table (tiny) and paints its 98-row
slice. The paint is a one-hot(segment) x seg_sums matmul on the tensor
engine, which directly produces the channel-major output layout. The 1/count
mean scaling is folded into the one-hot values, so the seg table is just the
raw bf16 segment sums. All matmuls run single-plane bf16: the 2e-2 rel-err
budget dwarfs bf16 rounding (~4e-3 end to end).
"""

import numpy as np
import ml_dtypes
from contextlib import ExitStack

import concourse.bass as bass
import concourse.tile as tile
from concourse import bacc, mybir
from concourse.bass_utils import run_bass_kernel_spmd

# problem shape (hardcoded per contract)
B, C, Hp, Wp = 2, 768, 28, 28
Hi, Wi = 392, 392
S = 128                    # segments per image
N_CORES = 8
ROWS = Hi // 4             # 98 full-res rows per core
NPIX = ROWS * Wi           # 38416 pixels per core
NPATCH = Hp * Wp           # 784 patch pixels
PCHUNK = 112               # 784 = 7 * 112 patch-pixel chunks (partition dim)
NCH = NPATCH // PCHUNK     # 7 chunks
PTILE = 512                # paint pixel tile (one PSUM bank)
GROUP = 3 * PTILE          # 1536 pixels per paint group
NGROUP = NPIX // GROUP     # 25 full groups
REM = NPIX - NGROUP * GROUP  # 16 remainder pixels
CT = C // 128              # 6 channel tiles

f32 = mybir.dt.float32
bf16 = mybir.dt.bfloat16
i32 = mybir.dt.int32

_CACHED_NC = None


def _build_nc():
    nc = bacc.Bacc()
    fpk_hbm = nc.dram_tensor("fpk", [PCHUNK, NCH, C], bf16, kind="ExternalInput")
    pmk_hbm = nc.dram_tensor("pmk", [PCHUNK, NCH], f32, kind="ExternalInput")
    mask_hbm = nc.dram_tensor("mask", [1, NPIX], bf16, kind="ExternalInput")
    out_hbm = nc.dram_tensor("out", [C, NPIX], f32, kind="ExternalOutput")

    with tile.TileContext(nc) as tc, ExitStack() as ctx:
        const = ctx.enter_context(tc.tile_pool(name="const", bufs=1))
        segp = ctx.enter_context(tc.tile_pool(name="segp", bufs=1))
        # paint-phase SBUF + one-hot PSUM pools created BEFORE the scatter
        # scratch pool so one-hot building can overlap the scatter phase
        sbB = ctx.enter_context(tc.tile_pool(name="sbB", bufs=6))
        osb = ctx.enter_context(tc.tile_pool(name="osb", bufs=10))
        psB = ctx.enter_context(tc.tile_pool(name="psB", bufs=2, space="PSUM"))

        # ---- input loads: pmk + feature chunks first (scatter critical
        # path), then the full-res mask (needed a bit later by the paint) ----
        ld = ctx.enter_context(tc.tile_pool(name="ld", bufs=1))
        pmk = ld.tile([PCHUNK, NCH], f32)
        nc.gpsimd.dma_start(out=pmk[:], in_=pmk_hbm[:, :])
        fsb = ld.tile([PCHUNK, NCH, C], bf16)
        # chunk 0 lands first so the first scatter matmul can start early
        nc.gpsimd.dma_start(out=fsb[:, 0:2, :], in_=fpk_hbm[:, 0:2, :])
        nc.gpsimd.dma_start(out=fsb[:, 2:NCH, :], in_=fpk_hbm[:, 2:NCH, :])
        mask_sb = ld.tile([1, NPIX], bf16)
        nc.gpsimd.dma_start(out=mask_sb[:], in_=mask_hbm[:, :])

        # ---- constants ----
        iota_pi = const.tile([128, 1], i32)           # partition index
        nc.gpsimd.iota(iota_pi[:], [[0, 1]], channel_multiplier=1)
        iota_pf = const.tile([128, 1], f32)
        nc.vector.tensor_copy(iota_pf[:], iota_pi[:])
        iota_ri = const.tile([128, 128], i32)         # free-dim index (same per partition)
        nc.gpsimd.iota(iota_ri[:], [[1, 128]], channel_multiplier=0)
        iota_rf = const.tile([128, 128], f32)
        nc.vector.tensor_copy(iota_rf[:], iota_ri[:])
        ones_bf = const.tile([1, 128], bf16)
        nc.vector.memset(ones_bf[:], 1.0)
        ones_col = const.tile([128, 1], bf16)
        nc.vector.memset(ones_col[:], 1.0)

        # ---- phase A: scatter-sum over patch pixels -> seg sums [S=128, C]
        # (the 1/count scaling is folded into the paint one-hots) ----
        seg_bf = segp.tile([128, C], bf16)
        rcp = segp.tile([128, 1], f32)

        psA_cm = tc.tile_pool(name="psA", bufs=1, space="PSUM")
        with tc.tile_pool(name="sbA", bufs=2) as sbA, psA_cm as psA:
            sums0 = psA.tile([128, 384], f32, tag="sums0", name="sums0")
            sums1 = psA.tile([128, 384], f32, tag="sums1", name="sums1")
            cnt_ps = psA.tile([128, 1], f32, tag="cnt", name="cnt")
            for k in range(NCH):
                oh = sbA.tile([PCHUNK, 128], bf16, tag="ohp")
                nc.vector.tensor_tensor(
                    out=oh[:], in0=iota_rf[0:PCHUNK, :],
                    in1=pmk[:, k:k + 1].to_broadcast([PCHUNK, 128]),
                    op=mybir.AluOpType.is_equal,
                )
                first, last = k == 0, k == NCH - 1
                nc.tensor.matmul(sums0[:], lhsT=oh[:], rhs=fsb[:, k, 0:384],
                                 start=first, stop=last)
                nc.tensor.matmul(sums1[:], lhsT=oh[:], rhs=fsb[:, k, 384:768],
                                 start=first, stop=last)
                nc.tensor.matmul(cnt_ps[:], lhsT=oh[:], rhs=ones_col[0:PCHUNK, :],
                                 start=first, stop=last)

            # rcp = 1 / max(cnt, 1); empty segments have sums == 0 so avg == 0
            cnt_sb = sbA.tile([128, 1], f32)
            nc.vector.tensor_scalar_max(cnt_sb[:], cnt_ps[:], 1.0)
            nc.vector.reciprocal(rcp[:], cnt_sb[:])
            # raw sums -> bf16 paint table
            nc.vector.tensor_copy(seg_bf[:, 0:384], sums0[:])
            nc.vector.tensor_copy(seg_bf[:, 384:768], sums1[:])

        # ---- phase B: paint full-res pixels ----
        psO = ctx.enter_context(tc.tile_pool(name="psO", bufs=6, space="PSUM"))

        def paint(pix0, sizes):
            # one group: pixels [pix0, pix0+sum(sizes)), one tile per size
            npx = sum(sizes)
            offs = [sum(sizes[:t]) for t in range(len(sizes))]
            ohs = []
            for t, sz in enumerate(sizes):
                bc = psB.tile([128, sz], f32, tag="bc", name="bc")
                nc.tensor.matmul(
                    bc[:], lhsT=ones_bf[:],
                    rhs=mask_sb[0:1, pix0 + offs[t]:pix0 + offs[t] + sz],
                    start=True, stop=True,
                )
                # one-hot scaled by 1/count: out = (bc == iota_p) * rcp
                oh = sbB.tile([128, sz], bf16, tag="ohb", name="ohb")
                nc.vector.tensor_scalar(
                    out=oh[:], in0=bc[:], scalar1=iota_pf[:], scalar2=rcp[:],
                    op0=mybir.AluOpType.is_equal, op1=mybir.AluOpType.mult,
                )
                ohs.append(oh)
            for c in range(CT):
                ob = osb.tile([128, npx], f32, tag="ob", name="ob")
                ops = [psO.tile([128, sz], f32, tag="op", name="op")
                       for sz in sizes]
                for t in range(len(sizes)):
                    nc.tensor.matmul(ops[t][:], lhsT=seg_bf[:, c * 128:(c + 1) * 128],
                                     rhs=ohs[t][:], start=True, stop=True)
                for t in range(len(sizes)):
                    # split psum->sbuf copies across DVE and ACT
                    dst = ob[:, offs[t]:offs[t] + sizes[t]]
                    if (c * len(sizes) + t) % 2 == 0:
                        nc.vector.tensor_copy(dst, ops[t][:])
                    else:
                        nc.scalar.copy(dst, ops[t][:])
                nc.sync.dma_start(
                    out=out_hbm[c * 128:(c + 1) * 128, pix0:pix0 + npx], in_=ob[:]
                )

        for g in range(NGROUP - 1):
            paint(g * GROUP, [PTILE] * 3)
        # last group absorbs the 16-pixel remainder as a 4th tile so the
        # final output DMA stays one large contiguous transfer per c-tile
        paint((NGROUP - 1) * GROUP, [PTILE] * 3 + ([REM] if REM else []))

    nc.compile()
    return nc


def make_in_maps(F_semantic_patches, segmentation_mask):
    F = np.asarray(F_semantic_patches, dtype=np.float32)
    M = np.asarray(segmentation_mask)
    in_maps = []
    for core in range(N_CORES):
        b, q = divmod(core, 4)
        feat = F[b].reshape(C, NPATCH).T                               # [784, 768]
        # [p, k, c] so one DMA lands chunk k on partitions
        fpk = np.ascontiguousarray(
            feat.reshape(NCH, PCHUNK, C).transpose(1, 0, 2)
        ).astype(ml_dtypes.bfloat16)
        pmk = np.ascontiguousarray(
            M[b, ::Hi // Hp, ::Wi // Wp].reshape(NCH, PCHUNK).T
        ).astype(np.float32)
        mask = np.ascontiguousarray(
            M[b, q * ROWS:(q + 1) * ROWS, :].reshape(1, NPIX)
        ).astype(ml_dtypes.bfloat16)
        in_maps.append({"fpk": fpk, "pmk": pmk, "mask": mask})
    return in_maps


def kernel(F_semantic_patches: np.ndarray, segmentation_mask: np.ndarray) -> np.ndarray:
    global _CACHED_NC
    if _CACHED_NC is None:
        _CACHED_NC = _build_nc()
    nc = _CACHED_NC

    in_maps = make_in_maps(F_semantic_patches, segmentation_mask)

    res = run_bass_kernel_spmd(nc, in_maps, core_ids=list(range(N_CORES)))

    out = np.empty((B, C, Hi, Wi), dtype=np.float32)
    for core in range(N_CORES):
        b, q = divmod(core, 4)
        out[b, :, q * ROWS:(q + 1) * ROWS, :] = (
            res.results[core]["out"].reshape(C, ROWS, Wi)
        )
    return out


# revision 5
# speedup vs baseline: 1.7022x; 1.6399x over previous
"""Trainium2 Bass kernel for nn_DinoGazeSpade (segment_reduce + repaint).

reference semantics:
  seg_feat = mask[:, ::14, ::14]                       # nearest-downsample to 28x28
  seg_avg[b, s, :] = mean of feat pixels with seg==s   # scatter_mean over B*128 segments
  out[b, :, hi, wi] = seg_avg[b, mask[b, hi, wi], :]   # repaint at full res

Sharding: 8 cores = 2 batches x 4 row-slices of the 392-row full-res output.
Each core computes its batch's seg table (tiny) and paints its 98-row
slice. The paint is a one-hot(segment) x seg_sums matmul on the tensor
engine, which directly produces the channel-major output layout. The 1/count
mean scaling is folded into the one-hot values, so the seg table is just the
raw bf16 segment sums. All matmuls run single-plane bf16: the 2e-2 rel-err
budget dwarfs bf16 rounding (~4e-3 end to end).
"""

import numpy as np
import ml_dtypes
from contextlib import ExitStack

import concourse.bass as bass
import concourse.tile as tile
from concourse import bacc, mybir
from concourse.bass_utils import run_bass_kernel_spmd

# problem shape (hardcoded per contract)
B, C, Hp, Wp = 2, 768, 28, 28
Hi, Wi = 392, 392
S = 128                    # segments per image
N_CORES = 8
ROWS = Hi // 4             # 98 full-res rows per core
NPIX = ROWS * Wi           # 38416 pixels per core
NPATCH = Hp * Wp           # 784 patch pixels
PCHUNK = 112               # 784 = 7 * 112 patch-pixel chunks (partition dim)
NCH = NPATCH // PCHUNK     # 7 chunks
PTILE = 512                # paint pixel tile (one PSUM bank)
GROUP = 3 * PTILE          # 1536 pixels per paint group
NGROUP = NPIX // GROUP     # 25 full groups
REM = NPIX - NGROUP * GROUP  # 16 remainder pixels
CT = C // 128              # 6 channel tiles

f32 = mybir.dt.float32
bf16 = mybir.dt.bfloat16
i32 = mybir.dt.int32

_CACHED_NC = None


def _build_nc():
    nc = bacc.Bacc()
    fpk_hbm = nc.dram_tensor("fpk", [PCHUNK, NCH, C], bf16, kind="ExternalInput")
    pmk_hbm = nc.dram_tensor("pmk", [PCHUNK, NCH], f32, kind="ExternalInput")
    mask_hbm = nc.dram_tensor("mask", [1, NPIX], bf16, kind="ExternalInput")
    out_hbm = nc.dram_tensor("out", [C, NPIX], bf16, kind="ExternalOutput")

    with tile.TileContext(nc) as tc, ExitStack() as ctx:
        const = ctx.enter_context(tc.tile_pool(name="const", bufs=1))
        segp = ctx.enter_context(tc.tile_pool(name="segp", bufs=1))
        # paint-phase SBUF + one-hot PSUM pools created BEFORE the scatter
        # scratch pool so one-hot building can overlap the scatter phase
        sbB = ctx.enter_context(tc.tile_pool(name="sbB", bufs=6))
        osb = ctx.enter_context(tc.tile_pool(name="osb", bufs=10))
        psB = ctx.enter_context(tc.tile_pool(name="psB", bufs=2, space="PSUM"))

        # ---- input loads: pmk + feature chunks first (scatter critical
        # path), then the full-res mask (needed a bit later by the paint) ----
        ld = ctx.enter_context(tc.tile_pool(name="ld", bufs=1))
        pmk = ld.tile([PCHUNK, NCH], f32)
        nc.gpsimd.dma_start(out=pmk[:], in_=pmk_hbm[:, :])
        fsb = ld.tile([PCHUNK, NCH, C], bf16)
        # chunk 0 lands first so the first scatter matmul can start early
        nc.gpsimd.dma_start(out=fsb[:, 0:2, :], in_=fpk_hbm[:, 0:2, :])
        nc.gpsimd.dma_start(out=fsb[:, 2:NCH, :], in_=fpk_hbm[:, 2:NCH, :])
        mask_sb = ld.tile([1, NPIX], bf16)
        nc.gpsimd.dma_start(out=mask_sb[:], in_=mask_hbm[:, :])

        # ---- constants ----
        iota_pi = const.tile([128, 1], i32)           # partition index
        nc.gpsimd.iota(iota_pi[:], [[0, 1]], channel_multiplier=1)
        iota_pf = const.tile([128, 1], f32)
        nc.vector.tensor_copy(iota_pf[:], iota_pi[:])
        iota_ri = const.tile([128, 128], i32)         # free-dim index (same per partition)
        nc.gpsimd.iota(iota_ri[:], [[1, 128]], channel_multiplier=0)
        iota_rf = const.tile([128, 128], f32)
        nc.vector.tensor_copy(iota_rf[:], iota_ri[:])
        ones_bf = const.tile([1, 128], bf16)
        nc.vector.memset(ones_bf[:], 1.0)
        ones_col = const.tile([128, 1], bf16)
        nc.vector.memset(ones_col[:], 1.0)

        # ---- phase A: scatter-sum over patch pixels -> seg sums [S=128, C]
        # (the 1/count scaling is folded into the paint one-hots) ----
        seg_bf = segp.tile([128, C], bf16)
        rcp = segp.tile([128, 1], f32)

        psA_cm = tc.tile_pool(name="psA", bufs=1, space="PSUM")
        with tc.tile_pool(name="sbA", bufs=2) as sbA, psA_cm as psA:
            sums0 = psA.tile([128, 384], f32, tag="sums0", name="sums0")
            sums1 = psA.tile([128, 384], f32, tag="sums1", name="sums1")
            cnt_ps = psA.tile([128, 1], f32, tag="cnt", name="cnt")
            for k in range(NCH):
                oh = sbA.tile([PCHUNK, 128], bf16, tag="ohp")
                nc.vector.tensor_tensor(
                    out=oh[:], in0=iota_rf[0:PCHUNK, :],
                    in1=pmk[:, k:k + 1].to_broadcast([PCHUNK, 128]),
                    op=mybir.AluOpType.is_equal,
                )
                first, last = k == 0, k == NCH - 1
                nc.tensor.matmul(sums0[:], lhsT=oh[:], rhs=fsb[:, k, 0:384],
                                 start=first, stop=last)
                nc.tensor.matmul(sums1[:], lhsT=oh[:], rhs=fsb[:, k, 384:768],
                                 start=first, stop=last)
                nc.tensor.matmul(cnt_ps[:], lhsT=oh[:], rhs=ones_col[0:PCHUNK, :],
                                 start=first, stop=last)

            # rcp = 1 / max(cnt, 1); empty segments have sums == 0 so avg == 0
            cnt_sb = sbA.tile([128, 1], f32)
            nc.vector.tensor_scalar_max(cnt_sb[:], cnt_ps[:], 1.0)
            nc.vector.reciprocal(rcp[:], cnt_sb[:])
            # raw sums -> bf16 paint table
            nc.vector.tensor_copy(seg_bf[:, 0:384], sums0[:])
            nc.vector.tensor_copy(seg_bf[:, 384:768], sums1[:])

        # ---- phase B: paint full-res pixels ----
        psO = ctx.enter_context(tc.tile_pool(name="psO", bufs=6, space="PSUM"))

        def paint(pix0, sizes):
            # one group: pixels [pix0, pix0+sum(sizes)), one tile per size
            npx = sum(sizes)
            offs = [sum(sizes[:t]) for t in range(len(sizes))]
            ohs = []
            for t, sz in enumerate(sizes):
                bc = psB.tile([128, sz], f32, tag="bc", name="bc")
                nc.tensor.matmul(
                    bc[:], lhsT=ones_bf[:],
                    rhs=mask_sb[0:1, pix0 + offs[t]:pix0 + offs[t] + sz],
                    start=True, stop=True,
                )
                # one-hot scaled by 1/count: out = (bc == iota_p) * rcp
                oh = sbB.tile([128, sz], bf16, tag="ohb", name="ohb")
                nc.vector.tensor_scalar(
                    out=oh[:], in0=bc[:], scalar1=iota_pf[:], scalar2=rcp[:],
                    op0=mybir.AluOpType.is_equal, op1=mybir.AluOpType.mult,
                )
                ohs.append(oh)
            for c in range(CT):
                ob = osb.tile([128, npx], bf16, tag="ob", name="ob")
                ops = [psO.tile([128, sz], f32, tag="op", name="op")
                       for sz in sizes]
                for t in range(len(sizes)):
                    nc.tensor.matmul(ops[t][:], lhsT=seg_bf[:, c * 128:(c + 1) * 128],
                                     rhs=ohs[t][:], start=True, stop=True)
                for t in range(len(sizes)):
                    # split psum->sbuf copies across DVE and ACT
                    dst = ob[:, offs[t]:offs[t] + sizes[t]]
                    if (c * len(sizes) + t) % 2 == 0:
                        nc.vector.tensor_copy(dst, ops[t][:])
                    else:
                        nc.scalar.copy(dst, ops[t][:])
                nc.sync.dma_start(
                    out=out_hbm[c * 128:(c + 1) * 128, pix0:pix0 + npx], in_=ob[:]
                )

        for g in range(NGROUP - 1):
            paint(g * GROUP, [PTILE] * 3)
        # last group absorbs the 16-pixel remainder as a 4th tile so the
        # final output DMA stays one large contiguous transfer per c-tile
        paint((NGROUP - 1) * GROUP, [PTILE] * 3 + ([REM] if REM else []))

    nc.compile()
    return nc


def make_in_maps(F_semantic_patches, segmentation_mask):
    F = np.asarray(F_semantic_patches, dtype=np.float32)
    M = np.asarray(segmentation_mask)
    in_maps = []
    for core in range(N_CORES):
        b, q = divmod(core, 4)
        feat = F[b].reshape(C, NPATCH).T                               # [784, 768]
        # [p, k, c] so one DMA lands chunk k on partitions
        fpk = np.ascontiguousarray(
            feat.reshape(NCH, PCHUNK, C).transpose(1, 0, 2)
        ).astype(ml_dtypes.bfloat16)
        pmk = np.ascontiguousarray(
            M[b, ::Hi // Hp, ::Wi // Wp].reshape(NCH, PCHUNK).T
        ).astype(np.float32)
        mask = np.ascontiguousarray(
            M[b, q * ROWS:(q + 1) * ROWS, :].reshape(1, NPIX)
        ).astype(ml_dtypes.bfloat16)
        in_maps.append({"fpk": fpk, "pmk": pmk, "mask": mask})
    return in_maps


def kernel(F_semantic_patches: np.ndarray, segmentation_mask: np.ndarray) -> np.ndarray:
    global _CACHED_NC
    if _CACHED_NC is None:
        _CACHED_NC = _build_nc()
    nc = _CACHED_NC

    in_maps = make_in_maps(F_semantic_patches, segmentation_mask)

    res = run_bass_kernel_spmd(nc, in_maps, core_ids=list(range(N_CORES)))

    out = np.empty((B, C, Hi, Wi), dtype=np.float32)
    for core in range(N_CORES):
        b, q = divmod(core, 4)
        out[b, :, q * ROWS:(q + 1) * ROWS, :] = (
            res.results[core]["out"].astype(np.float32).reshape(C, ROWS, Wi)
        )
    return out
